# Initial kernel scaffold
#
"""Trainium2 Bass kernel for nn_MessageAttentionPassing.

Math (reference):
    xp  = x.transpose(0,2,3,1)            # [B, N, T, CIN]
    h   = xp @ W1 + b1                    # [B, N, T, HID]
    mv  = h @ W2[:HID]                    # dest part
    mh  = h @ W2[HID:]                    # src part
    a    = attention[:, 0]                # [B, N(i), N(j), T]
    asum = a.sum(axis=2)                  # [B, N, T]
    upd = asum[...,None]*(mv+b2) + einsum('bijt,bjtc->bitc', a, mh)
    out = upd.transpose(0,3,1,2)          # [B, COUT, N, T]

Sharding: 8 cores = (batch b in {0,1}) x (dest-node quarter q in {0..3}).
Each core loads the full x[b] (h/mh need every source node j) plus its
32-row attention slice.  Inputs are node-rotated by i0 = 32*q on the host
so all cores run the IDENTICAL program (run_bass_kernel_spmd requires one
shared BIR program) with the core's dest chunk at rotated positions 0..31.

On-chip layouts (per core, rotated node axis n):
    xT2  [128, 1536]  partition=(half,cin), free=(n_local, t) n-major;
                      halves are node groups n<64 / n>=64
    hT2  [128, 1536]  partition=(half,hid), free=(t, n_local) t-major
    mh   [128, 1536]  partition=j (all 128 nodes), free=(t, c) t-major
    attT [128,  768]  partition=j, free=(i, t) i-major (transposed DMA load)
    per-t einsum: psum[c, t*32+i] = (lhsT=mh_t[j,c]) x (rhs=attT_t[j,i])

t is processed in TG groups of GT=8 so mh matmuls for group g start as
soon as group g of h is copied out of PSUM (clean contiguous-range deps).

The `reps`/`ablate` knobs exist only for benchmarking (the per-rep loop
amortizes the ~70 ms axon dispatch overhead; ablate drops stages).
"""

import os
import sys
import numpy as np

if "/opt/trn_rl_repo" not in sys.path:
    sys.path.insert(0, "/opt/trn_rl_repo")

B, CIN, N, T, COUT, HID = 2, 64, 128, 24, 64, 64
NI = N // 4          # dest-node chunk per core: 32
NT = N * T           # 3072
F2 = NT // 2         # 1536
IT = NI * T          # 768
GT = 8               # t-group size
TG = T // GT         # 3 groups
WCOLS = 194          # packed weight columns

_PROGRAM = None      # compiled program cache — compile once per process

_STAGE_OUT = {       # benchmark stage -> tile it writes (for ablation stubs)
    "xdma": ["xT2", "wpack"], "attdma": ["attT"], "h": ["hT2"],
    "mh": ["mh"], "mv": ["mvb2"], "asum": ["term1"], "fin": ["updT"],
    "einsum": ["ps_ae"],
}


def _build_program(reps: int = 1, ablate: frozenset = frozenset()):
    import concourse.bacc as bacc
    from concourse import mybir, tile

    f32 = mybir.dt.float32

    nc = bacc.Bacc(
        "TRN2",
        target_bir_lowering=False,
        debug=False,
        enable_asserts=False,
        num_devices=8,
    )

    dram = {
        "xr": nc.dram_tensor("xr", [CIN, NT], f32, kind="ExternalInput"),
        "att": nc.dram_tensor("att", [NI, N, T], f32, kind="ExternalInput"),
        "wpack": nc.dram_tensor("wpack", [128, WCOLS], f32,
                                kind="ExternalInput"),
        "out": nc.dram_tensor("out", [COUT, IT], f32, kind="ExternalOutput"),
    }

    with tile.TileContext(nc) as tc:
        with (
            tc.tile_pool(name="const", bufs=1) as cpool,
            tc.tile_pool(name="ps", bufs=1, space="PSUM") as pspool,
        ):
            tl = {}
            for nm, shape in (
                ("wpack", [128, WCOLS]), ("ones", [128, COUT]),
                ("xT2", [128, F2]), ("attT", [128, IT]), ("hT2", [128, F2]),
                ("mh", [128, F2]), ("mvb2", [COUT, IT]),
                ("term1", [COUT, IT]), ("updT", [COUT, IT]),
            ):
                tl[nm] = cpool.tile(shape, f32, name=nm)
            nc.vector.memset(tl["ones"][:], 1.0)
            tl["ps_w0"] = pspool.tile([128, 512], f32, name="ps_w0")
            tl["ps_w1"] = pspool.tile([128, 512], f32, name="ps_w1")
            tl["ps_m"] = pspool.tile([128, F2], f32, name="ps_m")
            tl["ps_ae"] = pspool.tile([COUT, IT], f32, name="ps_ae")

            # ablated producers: memset their outputs once so downstream
            # reads are backed by a written tile (benchmark-only path)
            for stage in ablate:
                for nm in _STAGE_OUT.get(stage, ()):
                    nc.vector.memset(tl[nm][:], 0.0)

            for _rep in range(reps):
                _rep_body(nc, tl, dram, mybir, ablate)

    nc.compile()
    return nc


def _rep_body(nc, tl, dram, mybir, ablate=frozenset()):
    from concourse.bass import ts

    Identity = mybir.ActivationFunctionType.Identity
    xT2, attT, hT2, mh = tl["xT2"], tl["attT"], tl["hT2"], tl["mh"]
    mvb2, term1, updT = tl["mvb2"], tl["term1"], tl["updT"]
    ps_m, ps_ae, wp = tl["ps_m"], tl["ps_ae"], tl["wpack"]
    w1t, wht = wp[:, 0:64], wp[:, 64:128]
    wvt = wp[0:64, 128:192]
    b1t, b2t = wp[:, 192:193], wp[0:64, 193:194]

    # ---- input DMAs.  x first (feeds the h->mh->einsum critical chain),
    # on the sync ring; packed weights on the scalar ring in parallel. ----
    if "xdma" not in ablate:
        nc.sync.dma_start(xT2[0:64, :], dram["xr"][:, 0:F2])
        nc.sync.dma_start(xT2[64:128, :], dram["xr"][:, F2:NT])
        nc.scalar.dma_start(tl["wpack"][:], dram["wpack"][:])
    # attention transposed load: partition=j, free=(i,t).  On the scalar
    # HWDGE ring so it runs concurrently with the x loads on the sync ring.
    if "attdma" not in ablate:
        nc.scalar.dma_start(attT[:],
                            dram["att"][:].rearrange("i j t -> j i t"))

    # hT is single-layout [64(hid), (n,t)]: mh then needs only ONE matmul
    # per t (M=128) — PE is issue-rate-bound (~100ns/matmul), so fewer,
    # larger matmuls win.  Strided stationary lhsT is cheap; strided
    # MOVING rhs with 2+ free dims is ~10x slower, so every rhs below is
    # contiguous or single-strided.
    if "h" not in ablate:
        for f in range(3):
            ps = tl["ps_w0"] if f % 2 == 0 else tl["ps_w1"]
            nc.tensor.matmul(
                ps[0:64, :], w1t[0:64, :], xT2[0:64, ts(f, 512)],
                start=True, stop=True,
            )
            nc.tensor.matmul(
                ps[64:128, :], w1t[64:128, :], xT2[64:128, ts(f, 512)],
                start=True, stop=True, tile_position=(64, 64),
            )
            nc.scalar.activation(
                hT2[:, ts(f, 512)], ps[:], Identity, bias=b1t,
            )

    # ---- mh in j-partition layout, free (t,c) t-major; einsum group g
    # fires as soon as PSUM bank g is copied to SBUF ----
    hT2v = hT2[:].rearrange("p (n t) -> p t n", t=T)     # [128, 24, 64]
    attTv = attT[:].rearrange("j (i t) -> j t i", t=T)   # [128, 24, 32]
    for g in range(TG):
        if "mh" not in ablate:
            for t in range(g * GT, (g + 1) * GT):
                sl = slice(t * 64, t * 64 + 64)
                nc.tensor.matmul(
                    ps_m[0:64, sl], hT2v[0:64, t, :], wht[0:64, :],
                    start=True, stop=True,
                )
                nc.tensor.matmul(
                    ps_m[64:128, sl], hT2v[64:128, t, :], wht[64:128, :],
                    start=True, stop=True, tile_position=(64, 64),
                )
            nc.vector.tensor_copy(mh[:, ts(g, 512)], ps_m[:, ts(g, 512)])

        # ---- einsum: ps_ae[c, t*32+i] = mh_t[j,c] x attT_t[j,i] ----
        if "einsum" not in ablate:
            for t in range(g * GT, (g + 1) * GT):
                nc.tensor.matmul(
                    ps_ae[:, t * NI:(t + 1) * NI], mh[:, ts(t, 64)],
                    attTv[:, t, :], start=True, stop=True,
                )

    # ---- mv+b2 for the dest chunk: rotated nodes 0..31 are exactly hT
    # cols [0, 768) — contiguous rhs, mvb2 (i,t)-major ----
    if "mv" not in ablate:
        for k, (off, sz) in enumerate(((0, 512), (512, 256))):
            ps = tl["ps_w1"] if k % 2 == 0 else tl["ps_w0"]
            nc.tensor.matmul(
                ps[0:64, 0:sz], wvt, hT2[0:64, off:off + sz],
                start=True, stop=True,
            )
            nc.scalar.activation(
                mvb2[:, off:off + sz], ps[0:64, 0:sz], Identity,
                bias=b2t,
            )

    # ---- asum broadcast over c: ones[j,c].T @ attT[j,(i,t)];
    # term1[c,(i,t)] = mvb2[c,(i,t)] * asum_bcast[c,(i,t)] ----
    if "asum" not in ablate:
        nc.tensor.matmul(tl["ps_w0"][0:64, 0:512], tl["ones"][:],
                         attT[:, 0:512], start=True, stop=True)
        nc.tensor.matmul(tl["ps_w1"][0:64, 0:256], tl["ones"][:],
                         attT[:, 512:768], start=True, stop=True)
        nc.vector.tensor_mul(term1[:, 0:512], mvb2[:, 0:512],
                             tl["ps_w0"][0:64, 0:512])
        nc.vector.tensor_mul(term1[:, 512:768], mvb2[:, 512:768],
                             tl["ps_w1"][0:64, 0:256])

    # ---- upd[c,(i,t)] = ps_ae[c,(t,i)] + term1[c,(i,t)]; store ----
    if "fin" not in ablate:
        pv = ps_ae[:].rearrange("c (t i) -> c i t", i=NI)
        t1v = term1[:].rearrange("c (i t) -> c i t", t=T)
        updTv = updT[:].rearrange("c (i t) -> c i t", t=T)
        nc.vector.tensor_add(updTv, pv, t1v)
    if "outdma" not in ablate:
        nc.sync.dma_start(dram["out"][:], updT[:])


def _get_program():
    global _PROGRAM
    if _PROGRAM is None:
        _PROGRAM = _build_program()
    return _PROGRAM


def _make_in_maps(x, attention, W1, b1, W2, b2):
    x = np.ascontiguousarray(x, dtype=np.float32)
    attention = np.ascontiguousarray(attention, dtype=np.float32)
    W1 = np.asarray(W1, dtype=np.float32)
    b1 = np.asarray(b1, dtype=np.float32)
    W2 = np.asarray(W2, dtype=np.float32)
    b2 = np.asarray(b2, dtype=np.float32)

    wpack = np.zeros((128, WCOLS), np.float32)
    wpack[:, 0:64] = np.concatenate([W1, W1], axis=0)
    wpack[:, 64:128] = np.concatenate([W2[HID:], W2[HID:]], axis=0)
    wpack[0:64, 128:192] = W2[:HID]
    wpack[:, 192] = np.concatenate([b1, b1])
    wpack[0:64, 193] = b2

    in_maps = []
    for k in range(8):
        b, q = k // 4, k % 4
        i0 = NI * q
        # rotate node axis so this core's dest chunk sits at positions 0..31;
        # j axis of the attention slice rotated identically to stay aligned.
        xb = np.ascontiguousarray(
            np.roll(x[b], -i0, axis=1).reshape(CIN, NT))
        att_c = np.ascontiguousarray(
            np.roll(attention[b, 0, i0:i0 + NI], -i0, axis=1))
        in_maps.append({"xr": xb, "att": att_c, "wpack": wpack})
    return in_maps


def run(inputs: dict, trace: bool = False):
    """Compile (cached), shard, run on 8 cores; returns (full_out, results)."""
    from concourse import bass_utils

    nc = _get_program()
    in_maps = _make_in_maps(**inputs)
    res = bass_utils.run_bass_kernel_spmd(
        nc, in_maps, core_ids=list(range(8)), trace=trace,
    )
    full = np.empty((B, COUT, N, T), dtype=np.float32)
    for k in range(8):
        b, q = k // 4, k % 4
        i0 = NI * q
        full[b, :, i0:i0 + NI, :] = res.results[k]["out"].reshape(COUT, NI, T)
    return full, res


def kernel(**inputs) -> np.ndarray:
    full, _ = run(inputs, trace=False)
    return full



# revision 1
# speedup vs baseline: 1.4431x; 1.4431x over previous
"""Trainium2 Bass kernel for nn_MessageAttentionPassing.

Math (reference):
    xp  = x.transpose(0,2,3,1)            # [B, N, T, CIN]
    h   = xp @ W1 + b1                    # [B, N, T, HID]
    mv  = h @ W2[:HID]                    # dest part
    mh  = h @ W2[HID:]                    # src part
    a    = attention[:, 0]                # [B, N(i), N(j), T]
    asum = a.sum(axis=2)                  # [B, N, T]
    upd = asum[...,None]*(mv+b2) + einsum('bijt,bjtc->bitc', a, mh)
    out = upd.transpose(0,3,1,2)          # [B, COUT, N, T]

Sharding: 8 cores = (batch b in {0,1}) x (dest-node quarter q in {0..3}).
Each core loads the full x[b] (h/mh need every source node j) plus its
32-row attention slice.  Inputs are node-rotated by i0 = 32*q on the host
so all cores run the IDENTICAL program (run_bass_kernel_spmd requires one
shared BIR program) with the core's dest chunk at rotated positions 0..31.

On-chip layouts (per core, rotated node axis n):
    xT2  [128, 1536]  partition=(half,cin), free=(n_local, t) n-major;
                      halves are node groups n<64 / n>=64
    hT2  [128, 1536]  partition=(half,hid), free=(t, n_local) t-major
    mh   [128, 1536]  partition=j (all 128 nodes), free=(t, c) t-major
    attT [128,  768]  partition=j, free=(i, t) i-major (transposed DMA load)
    per-t einsum: psum[c, t*32+i] = (lhsT=mh_t[j,c]) x (rhs=attT_t[j,i])

t is processed in TG groups of GT=8 so mh matmuls for group g start as
soon as group g of h is copied out of PSUM (clean contiguous-range deps).

The `reps`/`ablate` knobs exist only for benchmarking (the per-rep loop
amortizes the ~70 ms axon dispatch overhead; ablate drops stages).
"""

import os
import sys
import numpy as np

if "/opt/trn_rl_repo" not in sys.path:
    sys.path.insert(0, "/opt/trn_rl_repo")

B, CIN, N, T, COUT, HID = 2, 64, 128, 24, 64, 64
NI = N // 4          # dest-node chunk per core: 32
NT = N * T           # 3072
F2 = NT // 2         # 1536
IT = NI * T          # 768
GT = 8               # t-group size
TG = T // GT         # 3 groups
WCOLS = 194          # packed weight columns

_PROGRAM = None      # compiled program cache — compile once per process

_STAGE_OUT = {       # benchmark stage -> tile it writes (for ablation stubs)
    "xdma": ["xT2", "wpack"], "attdma": ["attT"], "h": ["hT2"],
    "mh": ["mh"], "mv": ["mvb2"], "asum": ["term1"], "fin": ["updT"],
    "einsum": ["ps_ae"],
}


def _build_program(reps: int = 1, ablate: frozenset = frozenset()):
    import concourse.bacc as bacc
    from concourse import mybir, tile

    f32 = mybir.dt.float32

    nc = bacc.Bacc(
        "TRN2",
        target_bir_lowering=False,
        debug=False,
        enable_asserts=False,
        num_devices=8,
    )

    dram = {
        "xr": nc.dram_tensor("xr", [CIN, NT], f32, kind="ExternalInput"),
        "att": nc.dram_tensor("att", [NI, N, T], f32, kind="ExternalInput"),
        "wpack": nc.dram_tensor("wpack", [128, WCOLS], f32,
                                kind="ExternalInput"),
        "out": nc.dram_tensor("out", [COUT, IT], f32, kind="ExternalOutput"),
    }

    with tile.TileContext(nc) as tc:
        with (
            tc.tile_pool(name="const", bufs=1) as cpool,
            tc.tile_pool(name="ps", bufs=1, space="PSUM") as pspool,
        ):
            tl = {}
            for nm, shape in (
                ("wpack", [128, WCOLS]), ("ones", [128, COUT]),
                ("xT2", [128, F2]), ("attT", [128, IT]), ("hT2", [128, F2]),
                ("mh", [128, F2]), ("mvb2", [COUT, IT]),
                ("term1", [COUT, IT]), ("updT", [COUT, IT]),
            ):
                tl[nm] = cpool.tile(shape, f32, name=nm)
            nc.vector.memset(tl["ones"][:], 1.0)
            tl["ps_w0"] = pspool.tile([128, 512], f32, name="ps_w0")
            tl["ps_w1"] = pspool.tile([128, 512], f32, name="ps_w1")
            tl["ps_m"] = pspool.tile([128, F2], f32, name="ps_m")
            tl["ps_ae"] = pspool.tile([COUT, IT], f32, name="ps_ae")

            # ablated producers: memset their outputs once so downstream
            # reads are backed by a written tile (benchmark-only path)
            for stage in ablate:
                for nm in _STAGE_OUT.get(stage, ()):
                    nc.vector.memset(tl[nm][:], 0.0)

            for _rep in range(reps):
                _rep_body(nc, tl, dram, mybir, ablate)

    nc.compile()
    return nc


def _rep_body(nc, tl, dram, mybir, ablate=frozenset()):
    from concourse.bass import ts

    Identity = mybir.ActivationFunctionType.Identity
    xT2, attT, hT2, mh = tl["xT2"], tl["attT"], tl["hT2"], tl["mh"]
    mvb2, term1, updT = tl["mvb2"], tl["term1"], tl["updT"]
    ps_m, ps_ae, wp = tl["ps_m"], tl["ps_ae"], tl["wpack"]
    w1t, wht = wp[:, 0:64], wp[:, 64:128]
    wvt = wp[0:64, 128:192]
    b1t, b2t = wp[:, 192:193], wp[0:64, 193:194]

    # ---- input DMAs.  x first (feeds the h->mh->einsum critical chain),
    # on the sync ring; packed weights on the scalar ring in parallel. ----
    if "xdma" not in ablate:
        nc.sync.dma_start(xT2[0:64, :], dram["xr"][:, 0:F2])
        nc.sync.dma_start(xT2[64:128, :], dram["xr"][:, F2:NT])
        nc.scalar.dma_start(tl["wpack"][:], dram["wpack"][:])
    # attention transposed load: partition=j, free=(i,t).  On the scalar
    # HWDGE ring so it runs concurrently with the x loads on the sync ring.
    if "attdma" not in ablate:
        nc.scalar.dma_start(attT[:],
                            dram["att"][:].rearrange("i j t -> j i t"))

    # hT is single-layout [64(hid), (n,t)]: mh then needs only ONE matmul
    # per t (M=128) — PE is issue-rate-bound (~100ns/matmul), so fewer,
    # larger matmuls win.  Strided stationary lhsT is cheap; strided
    # MOVING rhs with 2+ free dims is ~10x slower, so every rhs below is
    # contiguous or single-strided.
    if "h" not in ablate:
        for f in range(3):
            ps = tl["ps_w0"] if f % 2 == 0 else tl["ps_w1"]
            nc.tensor.matmul(
                ps[0:64, :], w1t[0:64, :], xT2[0:64, ts(f, 512)],
                start=True, stop=True,
            )
            nc.tensor.matmul(
                ps[64:128, :], w1t[64:128, :], xT2[64:128, ts(f, 512)],
                start=True, stop=True, tile_position=(64, 64),
            )
            nc.scalar.activation(
                hT2[:, ts(f, 512)], ps[:], Identity, bias=b1t,
            )

    # ---- mh in j-partition layout, free (t,c) t-major; einsum group g
    # fires as soon as PSUM bank g is copied to SBUF ----
    hT2v = hT2[:].rearrange("p (n t) -> p t n", t=T)     # [128, 24, 64]
    attTv = attT[:].rearrange("j (i t) -> j t i", t=T)   # [128, 24, 32]
    for g in range(TG):
        if "mh" not in ablate:
            for t in range(g * GT, (g + 1) * GT):
                sl = slice(t * 64, t * 64 + 64)
                nc.tensor.matmul(
                    ps_m[0:64, sl], hT2v[0:64, t, :], wht[0:64, :],
                    start=True, stop=True,
                )
                nc.tensor.matmul(
                    ps_m[64:128, sl], hT2v[64:128, t, :], wht[64:128, :],
                    start=True, stop=True, tile_position=(64, 64),
                )
            nc.vector.tensor_copy(mh[:, ts(g, 512)], ps_m[:, ts(g, 512)])

        # ---- einsum: ps_ae[c, t*32+i] = mh_t[j,c] x attT_t[j,i] ----
        if "einsum" not in ablate:
            for t in range(g * GT, (g + 1) * GT):
                nc.tensor.matmul(
                    ps_ae[:, t * NI:(t + 1) * NI], mh[:, ts(t, 64)],
                    attTv[:, t, :], start=True, stop=True,
                )

    # ---- mv+b2 for the dest chunk: rotated nodes 0..31 are exactly hT
    # cols [0, 768) — contiguous rhs, mvb2 (i,t)-major ----
    if "mv" not in ablate:
        for k, (off, sz) in enumerate(((0, 512), (512, 256))):
            ps = tl["ps_w1"] if k % 2 == 0 else tl["ps_w0"]
            nc.tensor.matmul(
                ps[0:64, 0:sz], wvt, hT2[0:64, off:off + sz],
                start=True, stop=True,
            )
            nc.scalar.activation(
                mvb2[:, off:off + sz], ps[0:64, 0:sz], Identity,
                bias=b2t,
            )

    # ---- asum broadcast over c: ones[j,c].T @ attT[j,(i,t)];
    # term1[c,(i,t)] = mvb2[c,(i,t)] * asum_bcast[c,(i,t)] ----
    if "asum" not in ablate:
        nc.tensor.matmul(tl["ps_w0"][0:64, 0:512], tl["ones"][:],
                         attT[:, 0:512], start=True, stop=True)
        nc.tensor.matmul(tl["ps_w1"][0:64, 0:256], tl["ones"][:],
                         attT[:, 512:768], start=True, stop=True)
        nc.vector.tensor_mul(term1[:, 0:512], mvb2[:, 0:512],
                             tl["ps_w0"][0:64, 0:512])
        nc.vector.tensor_mul(term1[:, 512:768], mvb2[:, 512:768],
                             tl["ps_w1"][0:64, 0:256])

    # ---- upd[c,(i,t)] = ps_ae[c,(t,i)] + term1[c,(i,t)]; store ----
    if "fin" not in ablate:
        pv = ps_ae[:].rearrange("c (t i) -> c i t", i=NI)
        t1v = term1[:].rearrange("c (i t) -> c i t", t=T)
        updTv = updT[:].rearrange("c (i t) -> c i t", t=T)
        nc.vector.tensor_add(updTv, pv, t1v)
    if "outdma" not in ablate:
        nc.sync.dma_start(dram["out"][:], updT[:])


def _get_program():
    global _PROGRAM
    if _PROGRAM is None:
        _PROGRAM = _build_program()
    return _PROGRAM


def _make_in_maps(x, attention, W1, b1, W2, b2):
    x = np.ascontiguousarray(x, dtype=np.float32)
    attention = np.ascontiguousarray(attention, dtype=np.float32)
    W1 = np.asarray(W1, dtype=np.float32)
    b1 = np.asarray(b1, dtype=np.float32)
    W2 = np.asarray(W2, dtype=np.float32)
    b2 = np.asarray(b2, dtype=np.float32)

    wpack = np.zeros((128, WCOLS), np.float32)
    wpack[:, 0:64] = np.concatenate([W1, W1], axis=0)
    wpack[:, 64:128] = np.concatenate([W2[HID:], W2[HID:]], axis=0)
    wpack[0:64, 128:192] = W2[:HID]
    wpack[:, 192] = np.concatenate([b1, b1])
    wpack[0:64, 193] = b2

    in_maps = []
    for k in range(8):
        b, q = k // 4, k % 4
        i0 = NI * q
        # rotate node axis so this core's dest chunk sits at positions 0..31;
        # j axis of the attention slice rotated identically to stay aligned.
        xb = np.ascontiguousarray(
            np.roll(x[b], -i0, axis=1).reshape(CIN, NT))
        att_c = np.ascontiguousarray(
            np.roll(attention[b, 0, i0:i0 + NI], -i0, axis=1))
        in_maps.append({"xr": xb, "att": att_c, "wpack": wpack})
    return in_maps


def run(inputs: dict, trace: bool = False):
    """Compile (cached), shard, run on 8 cores; returns (full_out, results)."""
    from concourse import bass_utils

    nc = _get_program()
    in_maps = _make_in_maps(**inputs)
    res = bass_utils.run_bass_kernel_spmd(
        nc, in_maps, core_ids=list(range(8)), trace=trace,
    )
    full = np.empty((B, COUT, N, T), dtype=np.float32)
    for k in range(8):
        b, q = k // 4, k % 4
        i0 = NI * q
        full[b, :, i0:i0 + NI, :] = res.results[k]["out"].reshape(COUT, NI, T)
    return full, res


def kernel(**inputs) -> np.ndarray:
    full, _ = run(inputs, trace=False)
    return full

